# revision 1
# baseline (speedup 1.0000x reference)
"""Trainium2 Bass kernel for nn_ContrastiveLearning (self-contained).

kernel(**inputs) takes the FULL unsharded inputs (as produced by the
problem's setup_inputs) and returns (logits_per_img, logits_per_depth),
each [4, 100, 100] fp32.

Sharding: 8 NeuronCores, core c = (batch b=c//2, modality m=c%2). Each core
streams its 26 MB feature map, computes conv1x1+ReLU -> patch MLP ->
LayerNorm -> eT [128,100], exchanges eT with its pair partner via a
2-core AllGather, and computes the 100x100 contrastive logits on-device.
"""
import numpy as np
import concourse.bass as bass
import concourse.bacc as bacc
import concourse.mybir as mybir
import concourse.tile as tile
from concourse.bass_utils import run_bass_kernel_spmd


F32 = mybir.dt.float32
F32R = mybir.dt.float32r
AF = mybir.ActivationFunctionType
ALU = mybir.AluOpType

NV = NH = 10          # patch grid
NP = NV * NH          # 100 patches
CPS = 16
ENC = 128
PIX = CPS * CPS       # 256 features per patch
LN_EPS = 1e-5
SLAB_PR = 2           # patch-rows per slab
N_SLABS = NV // SLAB_PR
SLAB_FREE = SLAB_PR * CPS * (NH * CPS)   # 2*16*160 = 5120 per chunk


def build_kernel(nc, conv_f32r=True, n_cores=8, exchange='cc'):
    cdt = F32R if conv_f32r else F32

    feat = nc.dram_tensor("feat", [2, 128, 160, 160], cdt, kind="ExternalInput")
    convw = nc.dram_tensor("convw", [128, 2], cdt, kind="ExternalInput")
    bias128 = nc.dram_tensor("bias128", [128, 1], F32, kind="ExternalInput")
    w1t = nc.dram_tensor("w1t", [128, 512], F32, kind="ExternalInput")
    w2t = nc.dram_tensor("w2t", [128, 256], F32, kind="ExternalInput")
    ln_g = nc.dram_tensor("ln_g", [128, 1], F32, kind="ExternalInput")
    ln_b = nc.dram_tensor("ln_b", [128, 1], F32, kind="ExternalInput")
    ls = nc.dram_tensor("ls", [1, 1], F32, kind="ExternalInput")
    ident = nc.dram_tensor("ident", [128, 128], F32, kind="ExternalInput")
    logits = nc.dram_tensor("logits", [NP, NP], F32, kind="ExternalOutput")

    if exchange == 'cc':
        cc_in = nc.dram_tensor("cc_in", [ENC, NP], F32)
        cc_out = nc.dram_tensor("cc_out", [2 * ENC, NP], F32)

    with tile.TileContext(nc) as tc:
        with (
            tc.tile_pool(name="slab", bufs=3) as slab_pool,
            tc.tile_pool(name="x1p", bufs=2) as x1_pool,
            tc.tile_pool(name="cst", bufs=1) as cst,
            tc.tile_pool(name="work", bufs=1) as work,
            tc.tile_pool(name="ps", bufs=2, space="PSUM") as ps,
        ):
            # constants / weights
            convw_s = cst.tile([128, 2], cdt, tag="convw")
            bias_s = cst.tile([128, 1], F32, tag="bias")
            w1t_s = cst.tile([128, 512], F32, tag="w1t")
            w2t_s = cst.tile([128, 256], F32, tag="w2t")
            g_s = cst.tile([128, 1], F32, tag="g")
            b_s = cst.tile([128, 1], F32, tag="b")
            ls_s = cst.tile([1, 1], F32, tag="ls")
            id_s = cst.tile([128, 128], F32, tag="id")
            ones_col = cst.tile([128, 1], F32, tag="onec")
            ones_row = cst.tile([1, 128], F32, tag="oner")
            for t, srct in ((convw_s, convw), (bias_s, bias128), (w1t_s, w1t),
                            (w2t_s, w2t), (g_s, ln_g), (b_s, ln_b), (ls_s, ls),
                            (id_s, ident)):
                nc.gpsimd.dma_start(t[:], srct[:])
            nc.gpsimd.memset(ones_col[:], 1.0)
            nc.gpsimd.memset(ones_row[:], 1.0)

            # exp(logit_scale) early (no data deps beyond ls)
            es_s = work.tile([1, 1], F32, tag="es")
            nc.scalar.activation(es_s[:], ls_s[:], AF.Exp)
            ps_es = ps.tile([128, 1], F32, tag="mm")
            nc.tensor.matmul(ps_es[:], ones_row[:], es_s[:], start=True, stop=True)
            s_col = work.tile([128, 1], F32, tag="scol")
            nc.vector.tensor_copy(s_col[:], ps_es[:])

            # conv: M=1 matmuls, 2 patches per MM (N=512), all to psum partition 0
            x_cmp = work.tile([NP, PIX], F32, tag="xc")
            for s in range(N_SLABS):
                x1 = x1_pool.tile([1, 4 * 5, PIX], F32, tag="x1")
                st = slab_pool.tile([128, 2, SLAB_PR * CPS, NH * CPS], cdt, tag="slab")
                stv = []
                for u in range(2):
                    nc.sync.dma_start(
                        st[:, u, :, :],
                        feat[u, :, s * SLAB_PR * CPS:(s + 1) * SLAB_PR * CPS, :],
                    )
                    # [128, (c:10), (h:32), (j:16)]
                    stv.append(st[:, u, :, :].rearrange("p h (c j) -> p c h j", c=NH))
                for k in range(10 * s, 10 * (s + 1)):  # groups of 2 patches
                    cvt = ps.tile([1, 2 * PIX], F32, tag="cv")
                    p = 2 * k
                    r, c = p // NH, p % NH
                    rho = r - s * SLAB_PR
                    for u in range(2):
                        rhs = stv[u][:, c:c + 2, rho * CPS:(rho + 1) * CPS, :]
                        nc.tensor.matmul(
                            cvt[0:1, :],
                            convw_s[:, u:u + 1],
                            rhs,
                            start=(u == 0), stop=(u == 1),
                        )
                    # evacuate + bias + relu, alternating engines
                    o = x1[0:1, 2 * (k - 10 * s):2 * (k - 10 * s) + 2, :]
                    if k % 2 == 0:
                        nc.scalar.activation(o, cvt[0:1, :], AF.Relu,
                                             bias=bias_s[0:1, :])
                    else:
                        nc.vector.tensor_scalar(o, cvt[0:1, :], bias_s[0:1, :],
                                                0.0, ALU.add, ALU.max)

                # compact x1 -> x_cmp rows [20s, 20s+20)
                nc.scalar.dma_start(x_cmp[20 * s:20 * (s + 1), :], x1[0:1, :, :])

            # transpose -> xT chunks [128, 100]
            xT = []
            for u in range(2):
                pt = ps.tile([128, NP], F32, tag="mm")
                nc.tensor.transpose(pt[:], x_cmp[:, u * 128:(u + 1) * 128], id_s[0:NP, 0:NP])
                t = work.tile([128, NP], F32, tag=f"xT{u}")
                nc.vector.tensor_copy(t[:], pt[:])
                xT.append(t)

            # MLP layer 1: hT_v = relu(sum_u w1T(u,v) @ xT_u)
            hT = []
            for v in range(2):
                ph = ps.tile([128, NP], F32, tag="mm")
                for u in range(2):
                    nc.tensor.matmul(
                        ph[:], w1t_s[:, 256 * u + 128 * v:256 * u + 128 * v + 128],
                        xT[u][:], start=(u == 0), stop=(u == 1),
                    )
                t = work.tile([128, NP], F32, tag=f"hT{v}")
                nc.scalar.activation(t[:], ph[:], AF.Relu)
                hT.append(t)

            # MLP layer 2: yT = sum_u w2T(u) @ hT_u
            py = ps.tile([128, NP], F32, tag="mm")
            for u in range(2):
                nc.tensor.matmul(
                    py[:], w2t_s[:, 128 * u:128 * u + 128], hT[u][:],
                    start=(u == 0), stop=(u == 1),
                )
            yT = work.tile([128, NP], F32, tag="yT")
            nc.vector.tensor_copy(yT[:], py[:])

            # LayerNorm over partition dim via ones-matmul stats
            sq = work.tile([128, NP], F32, tag="sq")
            nc.vector.tensor_tensor(sq[:], yT[:], yT[:], ALU.mult)
            pmu = ps.tile([1, NP], F32, tag="mm")
            psq = ps.tile([1, NP], F32, tag="mm")
            nc.tensor.matmul(pmu[:], ones_col[:], yT[:], start=True, stop=True)
            nc.tensor.matmul(psq[:], ones_col[:], sq[:], start=True, stop=True)
            mrow = work.tile([1, NP], F32, tag="mrow")
            nc.vector.tensor_scalar_mul(mrow[:], pmu[:], 1.0 / ENC)
            qrow = work.tile([1, NP], F32, tag="qrow")
            nc.vector.tensor_scalar_mul(qrow[:], psq[:], 1.0 / ENC)
            m2 = work.tile([1, NP], F32, tag="m2")
            nc.vector.tensor_tensor(m2[:], mrow[:], mrow[:], ALU.mult)
            veps = work.tile([1, NP], F32, tag="veps")
            nc.vector.tensor_tensor(veps[:], qrow[:], m2[:], ALU.subtract)
            nc.vector.tensor_scalar_add(veps[:], veps[:], LN_EPS)
            srow = work.tile([1, NP], F32, tag="srow")
            nc.scalar.activation(srow[:], veps[:], AF.Sqrt)
            rrow = work.tile([1, NP], F32, tag="rrow")
            nc.vector.reciprocal(rrow[:], srow[:])
            # one Newton polish step: r' = r * (1.5 - 0.5*v*r^2)
            t1 = work.tile([1, NP], F32, tag="t1")
            nc.vector.tensor_tensor(t1[:], rrow[:], rrow[:], ALU.mult)
            nc.vector.tensor_tensor(t1[:], t1[:], veps[:], ALU.mult)
            nc.vector.tensor_scalar(t1[:], t1[:], -0.5, 1.5, ALU.mult, ALU.add)
            nc.vector.tensor_tensor(rrow[:], rrow[:], t1[:], ALU.mult)
            # nmr = -mean * rstd
            nmr = work.tile([1, NP], F32, tag="nmr")
            nc.vector.tensor_tensor(nmr[:], mrow[:], rrow[:], ALU.mult)
            nc.vector.tensor_scalar_mul(nmr[:], nmr[:], -1.0)
            # broadcast rows across partitions via K=1 matmul
            pA = ps.tile([128, NP], F32, tag="bc")
            pB = ps.tile([128, NP], F32, tag="bc")
            nc.tensor.matmul(pA[:], ones_row[:], rrow[:], start=True, stop=True)
            nc.tensor.matmul(pB[:], ones_row[:], nmr[:], start=True, stop=True)
            yn = work.tile([128, NP], F32, tag="yn")
            nc.vector.tensor_tensor(yn[:], yT[:], pA[:], ALU.mult)
            nc.vector.tensor_tensor(yn[:], yn[:], pB[:], ALU.add)
            eT = work.tile([128, NP], F32, tag="eT")
            nc.vector.tensor_scalar(eT[:], yn[:], g_s[:], b_s[:], ALU.mult, ALU.add)

            # exchange eT within pairs
            A_s = work.tile([128, NP], F32, tag="A")
            B_s = work.tile([128, NP], F32, tag="B")
            if exchange == 'rdma':
                rsem = nc.alloc_semaphore("rdma_rsem")
                lsem = nc.alloc_semaphore("rdma_lsem")
                nc.gpsimd.remote_dma_broadcast(
                    B_s[:], eT[:], remote_sem=rsem, local_sem=lsem,
                    rdests=[(0, 1)] + [None] * 7)
                nc.gpsimd.trigger_dma(count=None)
                nc.vector.tensor_copy(A_s[:], eT[:])
                with tc.tile_critical():
                    nc.vector.wait_ge(rsem, 2)
                    nc.vector.tensor_copy(B_s[:], B_s[:])
            elif exchange == 'cc':
                nc.scalar.dma_start(cc_in[:], eT[:])
                groups = [[2 * i, 2 * i + 1] for i in range(n_cores // 2)]
                nc.gpsimd.collective_compute(
                    "AllGather", ALU.bypass, replica_groups=groups,
                    ins=[cc_in.ap().opt()], outs=[cc_out.ap().opt()],
                )
                nc.scalar.dma_start(A_s[:], cc_out[0:ENC, :])
                nc.scalar.dma_start(B_s[:], cc_out[ENC:2 * ENC, :])
            else:
                nc.vector.tensor_copy(A_s[:], eT[:])
                nc.vector.tensor_copy(B_s[:], eT[:])

            # logits = (s*A).T @ B
            As2 = work.tile([128, NP], F32, tag="As2")
            nc.vector.tensor_scalar_mul(As2[:], A_s[:], s_col[:])
            pL = ps.tile([NP, NP], F32, tag="bc")
            nc.tensor.matmul(pL[:], As2[:], B_s[:], start=True, stop=True)
            L_s = work.tile([NP, NP], F32, tag="Ls")
            nc.vector.tensor_copy(L_s[:], pL[:])
            nc.scalar.dma_start(logits[:], L_s[:])

    nc.compile()
    return nc


def host_inputs_for_core(core, inputs):
    """Build the per-core in_map from the full problem inputs dict."""
    b, m = core // 2, core % 2
    feat = np.asarray(inputs["feat_c1" if m == 0 else "feat_c2"])[b]
    pre = "img_" if m == 0 else "depth_"
    cw = np.asarray(inputs[pre + "conv_w"]).reshape(2, 128).T.copy()   # [128,2]
    cb = np.full((128, 1), np.asarray(inputs[pre + "conv_b"])[0], np.float32)
    w1 = np.asarray(inputs[pre + "w1"])  # [256,256] (o=128v+m', i=128u+k)
    w1t = np.ascontiguousarray(
        w1.reshape(2, 128, 2, 128).transpose(3, 2, 0, 1).reshape(128, 512))
    w2 = np.asarray(inputs[pre + "w2"])  # [128,256]
    w2t = np.ascontiguousarray(
        w2.reshape(128, 2, 128).transpose(2, 1, 0).reshape(128, 256))
    return {
        "feat": np.ascontiguousarray(feat).reshape(2, 128, 160, 160),
        "convw": cw.astype(np.float32),
        "bias128": cb,
        "w1t": w1t.astype(np.float32),
        "w2t": w2t.astype(np.float32),
        "ln_g": np.asarray(inputs[pre + "ln_g"]).reshape(128, 1).astype(np.float32),
        "ln_b": np.asarray(inputs[pre + "ln_b"]).reshape(128, 1).astype(np.float32),
        "ls": np.asarray(inputs["logit_scale"]).reshape(1, 1).astype(np.float32),
        "ident": np.eye(128, dtype=np.float32),
    }


_NC_CACHE = {}


def _get_nc():
    if "nc" not in _NC_CACHE:
        nc = bacc.Bacc("TRN2", target_bir_lowering=False, num_devices=8)
        build_kernel(nc, conv_f32r=True, n_cores=8, exchange="cc")
        _NC_CACHE["nc"] = nc
    return _NC_CACHE["nc"]


def kernel(**inputs):
    nc = _get_nc()
    in_maps = [host_inputs_for_core(c, inputs) for c in range(8)]
    res = run_bass_kernel_spmd(nc, in_maps, list(range(8)))
    logits_img = np.stack([np.asarray(res.results[2 * b]["logits"])
                           for b in range(4)]).astype(np.float32)
    logits_depth = np.ascontiguousarray(logits_img.transpose(0, 2, 1))
    return logits_img, logits_depth



# revision 2
# speedup vs baseline: 1.0697x; 1.0697x over previous
"""Trainium2 Bass kernel for nn_ContrastiveLearning (self-contained), v2.

kernel(**inputs) takes the FULL unsharded inputs and returns
(logits_per_img, logits_per_depth), each [4, 100, 100] fp32.

Sharding: 8 NeuronCores, core c = (batch b=c//2, modality m=c%2). Each core
streams its feature map as bf16 (13.1 MB), computes conv1x1+ReLU -> patch
MLP -> LayerNorm pipelined per 2-patch-row slab, exchanges eT with its pair
partner via a 2-core AllGather, and computes the 100x100 logits on-device.

The 1x1 conv uses the feature tile as the PE *stationary* operand
([128ch x 128pix] per half-patch) and the conv weight as the moving operand,
so each matmul emits one [128pix, 1] column directly in the transposed
layout the MLP needs — no separate transpose, no partition-scatter DMA.
exp(logit_scale) is folded into the img branch's LayerNorm gain/bias on the
host.
"""
import numpy as np
import ml_dtypes
import concourse.bass as bass
import concourse.bacc as bacc
import concourse.mybir as mybir
import concourse.tile as tile
from concourse.bass_utils import run_bass_kernel_spmd


F32 = mybir.dt.float32
BF16 = mybir.dt.bfloat16
FP8 = mybir.dt.float8e4
FEAT_FP8 = False
FEAT_DT = FP8 if FEAT_FP8 else BF16
FEAT_NP = ml_dtypes.float8_e4m3 if FEAT_FP8 else ml_dtypes.bfloat16
AF = mybir.ActivationFunctionType
ALU = mybir.AluOpType

NV = NH = 10          # patch grid
NP = NV * NH          # 100 patches
CPS = 16
ENC = 128
PIX = CPS * CPS       # 256 features per patch
LN_EPS = 1e-5
SLAB_PR = 2           # patch-rows per slab
N_SLABS = NV // SLAB_PR
SLAB_H = SLAB_PR * CPS        # 32 rows
W = NH * CPS                  # 160 cols
COLS = SLAB_PR * NH           # 20 patches (eT columns) per slab
HHALF = CPS // 2              # 8 patch-rows per 128-pixel half


def build_kernel(nc, n_cores=8, exchange='cc'):
    # host pre-arranges features to [u, channel, patch, pixel] (patch-major,
    # pixel-contiguous) so conv stationary slices are single-stride
    feat = nc.dram_tensor("feat", [2, 128, NP, PIX], FEAT_DT, kind="ExternalInput")
    convw = nc.dram_tensor("convw", [128, 2], FEAT_DT, kind="ExternalInput")
    # packed1: cols 0:512 w1t, 512:768 w2t (bf16)
    packed1 = nc.dram_tensor("packed1", [128, 768], BF16, kind="ExternalInput")
    bias128 = nc.dram_tensor("bias128", [128, 1], F32, kind="ExternalInput")
    # rows: -g_eff, b_eff, g_eff (scale folded in on host for the img branch)
    gbr = nc.dram_tensor("gbr", [3, 128], F32, kind="ExternalInput")
    logits = nc.dram_tensor("logits", [NP, NP], F32, kind="ExternalOutput")

    if exchange == 'cc':
        cc_in = nc.dram_tensor("cc_in", [ENC, NP], F32)
        cc_out = nc.dram_tensor("cc_out", [2 * ENC, NP], F32)

    with tile.TileContext(nc) as tc:
        with (
            tc.tile_pool(name="slab", bufs=3) as slab_pool,
            tc.tile_pool(name="cst", bufs=1) as cst,
            tc.tile_pool(name="work", bufs=1) as work,
            tc.tile_pool(name="pcv", bufs=2, space="PSUM") as pcv,
            tc.tile_pool(name="ps", bufs=1, space="PSUM") as ps,
        ):
            # ---- slab 0 / u=0 chunk first (the stream is the critical
            # path), then the constants, then the remaining chunks
            st0 = slab_pool.tile([128, 2, COLS, PIX], FEAT_DT, tag="slab")
            nc.sync.dma_start(st0[:, 0, :, :], feat[0, :, 0:COLS, :])

            convw_s = cst.tile([128, 2], FEAT_DT, tag="convw")
            p1_s = cst.tile([128, 768], BF16, tag="p1")
            bias_s = cst.tile([128, 1], F32, tag="bias")
            gbA = cst.tile([2, 128], F32, tag="gbA")
            gbB = cst.tile([1, 128], F32, tag="gbB")
            nc.sync.dma_start(convw_s[:], convw[:])
            nc.sync.dma_start(bias_s[:], bias128[:])
            nc.sync.dma_start(gbA[:], gbr[0:2, :])
            nc.sync.dma_start(gbB[:], gbr[2:3, :])
            nc.sync.dma_start(p1_s[:], packed1[:])
            w1t_s = p1_s[:, 0:512]
            w2t_s = p1_s[:, 512:768]

            ones_col = cst.tile([128, 1], F32, tag="onec")
            nc.gpsimd.memset(ones_col[:], 1.0)
            eps_s = cst.tile([1, 1], F32, tag="eps")
            nc.gpsimd.memset(eps_s[:], LN_EPS)
            # R2 row1 stays ones (for the rank-2 additive broadcast);
            # row0 is overwritten per slab with mean*rstd
            R2 = work.tile([2, NP], F32, tag="R2")
            nc.gpsimd.memset(R2[:], 1.0)

            xT0 = work.tile([128, NP], BF16, tag="xT0")
            xT1 = work.tile([128, NP], BF16, tag="xT1")
            xT = [xT0, xT1]
            ymix = work.tile([128, 2 * COLS], F32, tag="ymix")
            rr = work.tile([1, NP], F32, tag="rr")
            eT = work.tile([128, NP], F32, tag="eT")

            for s in range(N_SLABS):
                st = st0 if s == 0 else slab_pool.tile(
                    [128, 2, COLS, PIX], FEAT_DT, tag="slab")
                for u in range(2):
                    if not (s == 0 and u == 0):
                        nc.sync.dma_start(
                            st[:, u, :, :],
                            feat[u, :, s * COLS:(s + 1) * COLS, :],
                        )

                # conv: per (u-chunk, pixel-half h, patch l) one [128pix, 1]
                # column; u=0 and u=1 go to separate psum columns (summed at
                # eviction) so every matmul is a complete group and all u=0
                # work overlaps the u=1 chunk's DMA
                cvt = pcv.tile([128, 4 * COLS], F32, tag="cv")
                for u in range(2):
                    for h in range(2):
                        for l in range(COLS):
                            lhsT = st[:, u, l, h * 128:(h + 1) * 128]
                            col = 2 * COLS * u + COLS * h + l
                            nc.tensor.matmul(
                                cvt[:, col:col + 1],
                                lhsT, convw_s[:, u:u + 1],
                                start=True, stop=True,
                            )
                # evict: x = relu(u0 + u1 + bias), straight into the xT layout
                c0 = COLS * s
                tu1 = work.tile([128, 2 * COLS], F32, tag="tu1")
                nc.vector.tensor_copy(tu1[:], cvt[:, 2 * COLS:4 * COLS])
                x01 = work.tile([128, 2 * COLS], F32, tag="x01")
                nc.vector.scalar_tensor_tensor(
                    x01[:], cvt[:, 0:2 * COLS], bias_s, tu1[:],
                    ALU.add, ALU.add)
                nc.scalar.activation(xT0[:, c0:c0 + COLS], x01[:, 0:COLS],
                                     AF.Relu)
                nc.gpsimd.tensor_scalar_max(xT1[:, c0:c0 + COLS],
                                            x01[:, COLS:2 * COLS], 0.0)

                csl = slice(c0, c0 + COLS)
                # MLP layer 1 + ReLU
                hT = []
                for v in range(2):
                    ph = ps.tile([128, COLS], F32, tag=f"mm{v}")
                    for u in range(2):
                        nc.tensor.matmul(
                            ph[:], w1t_s[:, 256 * u + 128 * v:
                                         256 * u + 128 * v + 128],
                            xT[u][:, csl], start=(u == 0), stop=(u == 1))
                    t = work.tile([128, COLS], BF16, tag=f"hT{v}")
                    if v == 0:
                        nc.scalar.activation(t[:], ph[:], AF.Relu)
                    else:
                        nc.vector.tensor_scalar(t[:], ph[:], 0.0, 0.0,
                                                ALU.add, ALU.max)
                    hT.append(t)
                # MLP layer 2
                py = ps.tile([128, COLS], F32, tag="my")
                for u in range(2):
                    nc.tensor.matmul(py[:], w2t_s[:, 128 * u:128 * u + 128],
                                     hT[u][:], start=(u == 0), stop=(u == 1))
                nc.vector.tensor_copy(ymix[:, 0:COLS], py[:])
                nc.scalar.activation(ymix[:, COLS:2 * COLS], py[:], AF.Square)

                # LayerNorm stats: one matmul gives [sum(y) | sum(y^2)]
                pmix = ps.tile([1, 2 * COLS], F32, tag="st")
                nc.tensor.matmul(pmix[:], ones_col[:], ymix[:],
                                 start=True, stop=True)
                # m2 = (sum(y)/128)^2, vm = sum(y^2)/128 - m2,
                # srow = sqrt(vm + eps), rr = 1/srow, R2row0 = mean*rstd
                m2 = work.tile([1, COLS], F32, tag="m2")
                nc.scalar.activation(m2[:], pmix[0:1, 0:COLS], AF.Square,
                                     scale=1.0 / ENC)
                vm = work.tile([1, COLS], F32, tag="vm")
                nc.vector.scalar_tensor_tensor(
                    vm[:], pmix[0:1, COLS:2 * COLS], 1.0 / ENC, m2[:],
                    ALU.mult, ALU.subtract)
                srow = work.tile([1, COLS], F32, tag="srow")
                nc.scalar.activation(srow[:], vm[:], AF.Sqrt, bias=eps_s[:])
                nc.vector.reciprocal(rr[0:1, csl], srow[:])
                nc.vector.scalar_tensor_tensor(
                    R2[0:1, csl], pmix[0:1, 0:COLS], 1.0 / ENC, rr[0:1, csl],
                    ALU.mult, ALU.mult)
                # rank-1 / rank-2 broadcasts via outer-product matmuls
                pmul = ps.tile([128, COLS], F32, tag="bc0")
                padd = ps.tile([128, COLS], F32, tag="bc1")
                nc.tensor.matmul(pmul[:], gbB[:], rr[0:1, csl],
                                 start=True, stop=True)
                nc.tensor.matmul(padd[:], gbA[:], R2[:, csl],
                                 start=True, stop=True)
                t1 = work.tile([128, COLS], F32, tag="t1")
                nc.vector.tensor_tensor(t1[:], ymix[:, 0:COLS], pmul[:],
                                        ALU.mult)
                nc.vector.tensor_tensor(eT[:, csl], t1[:], padd[:], ALU.add)
                if exchange == 'cc':
                    nc.scalar.dma_start(cc_in[:, csl], eT[:, csl])

            # exchange eT within pairs
            B_s = work.tile([128, NP], F32, tag="B")
            if exchange == 'cc':
                groups = [[2 * i, 2 * i + 1] for i in range(n_cores // 2)]
                nc.gpsimd.collective_compute(
                    "AllGather", ALU.bypass, replica_groups=groups,
                    ins=[cc_in.ap().opt()], outs=[cc_out.ap().opt()],
                )
                nc.scalar.dma_start(B_s[:], cc_out[ENC:2 * ENC, :])
            else:
                nc.vector.tensor_copy(B_s[:], eT[:])

            # logits = eT.T @ B  (scale already folded into img g/b)
            pL = ps.tile([NP, NP], F32, tag="st")
            nc.tensor.matmul(pL[:], eT[:], B_s[:], start=True, stop=True)
            L_s = work.tile([NP, NP], F32, tag="Ls")
            nc.vector.tensor_copy(L_s[:], pL[:])
            nc.scalar.dma_start(logits[:], L_s[:])

    nc.compile()
    return nc


def host_inputs_for_core(core, inputs):
    """Build the per-core in_map from the full problem inputs dict."""
    b, m = core // 2, core % 2
    feat = np.asarray(inputs["feat_c1" if m == 0 else "feat_c2"])[b]
    pre = "img_" if m == 0 else "depth_"
    cw = np.asarray(inputs[pre + "conv_w"]).reshape(2, 128).T
    w1 = np.asarray(inputs[pre + "w1"])  # [256,256] (o=128v+m', i=128u+k)
    w1t = w1.reshape(2, 128, 2, 128).transpose(3, 2, 0, 1).reshape(128, 512)
    w2 = np.asarray(inputs[pre + "w2"])  # [128,256]
    w2t = w2.reshape(128, 2, 128).transpose(2, 1, 0).reshape(128, 256)
    packed1 = np.concatenate([w1t, w2t], axis=1)

    scale = (np.exp(np.float32(np.asarray(inputs["logit_scale"])))
             if m == 0 else np.float32(1.0))
    g_eff = (np.asarray(inputs[pre + "ln_g"]) * scale).astype(np.float32)
    b_eff = (np.asarray(inputs[pre + "ln_b"]) * scale).astype(np.float32)
    gbr = np.stack([-g_eff, b_eff, g_eff]).astype(np.float32)
    bias = np.full((128, 1), np.asarray(inputs[pre + "conv_b"])[0], np.float32)

    fr = (feat.reshape(2, 128, 10, 16, 10, 16)
              .transpose(0, 1, 2, 4, 3, 5).reshape(2, 128, NP, PIX))
    return {
        "feat": np.ascontiguousarray(fr).astype(FEAT_NP),
        "convw": cw.astype(FEAT_NP),
        "packed1": np.ascontiguousarray(packed1).astype(ml_dtypes.bfloat16),
        "bias128": bias,
        "gbr": np.ascontiguousarray(gbr),
    }


_NC_CACHE = {}


def _get_nc():
    if "nc" not in _NC_CACHE:
        nc = bacc.Bacc("TRN2", target_bir_lowering=False, num_devices=8)
        build_kernel(nc, n_cores=8, exchange="cc")
        _NC_CACHE["nc"] = nc
    return _NC_CACHE["nc"]


def kernel(**inputs):
    nc = _get_nc()
    in_maps = [host_inputs_for_core(c, inputs) for c in range(8)]
    res = run_bass_kernel_spmd(nc, in_maps, list(range(8)))
    logits_img = np.stack(
        [np.asarray(res.results[2 * b]["logits"]) for b in range(4)]
    ).astype(np.float32)
    logits_depth = np.ascontiguousarray(logits_img.transpose(0, 2, 1))
    return logits_img, logits_depth
